# revision 6
# baseline (speedup 1.0000x reference)
"""EHR memory-network kernel for Trainium2 (8 NeuronCores, data-parallel over batch).

Reformulation of the reference scatter-scan:
  For patient b the scan applies, per event e (in time order), the affine update
      M[id_e] = M[id_e] * Af[e] + Bf[e]
  Since slot 0 is never touched (ids >= 1) and every touched slot starts from the
  same init_mem vector, the final row for node n is
      M[n] = init_mem * A_tot[n] + B_tot[n]
  with A_tot[n] = prod_{e: id_e=n} Af[e],  B_tot[n] = sum_{e: id_e=n} Bf[e]*SufA[e],
  SufA[e] = prod_{j>e, id_j=id_e} Af[j].  Af entries lie in (0,1] so products are
  exp(sum ln Af) and the id-grouped sums become matmuls against on-device compare
  matrices G[j,e] = (id_j == id_e) (strict-lower-triangle-masked for the suffix).

Device responsibilities (all value math): gate matmuls + tanh, D-level chain
composition (partition shifts done on the PE via shift-identity matmuls with
host-shifted coefficient vectors - no SBUF->SBUF DMA), ln/exp + G matmuls, the
full 16MB init-table write, and a dense per-event row buffer.  Host prep/finish
is index-only: valid-(t,mod) compaction, load-balanced patient->core assignment,
gather index lists, weight repacking, and final row placement out[id_e] = row[e]
(every event of a node carries the identical final row, so order is irrelevant).
sigmoid is computed as (1+tanh(z/2))/2 folded into per-partition scale vectors.
"""

import numpy as np
from contextlib import ExitStack

import concourse.bass as bass
import concourse.tile as tile
from concourse import bacc, mybir
from concourse import bass_utils

F32 = mybir.dt.float32
R32 = mybir.dt.float32r
I32 = mybir.dt.int32
AF = mybir.ActivationFunctionType
OP = mybir.AluOpType

# Problem shapes (hardcoded per contest contract).
B, T, MOD, D = 32, 64, 3, 4
WD, MEM, HID, DEMO = 256, 256, 512, 64
N_NODES = 4096
N_CORES = 8
BP = B // N_CORES              # patient slots per core = 4
NCH = 7                        # event chunks of 128 per core
P = 128
S_C = NCH * P                  # events per core = 896 (224 (t,mod) groups)
X_ROWS = BP * T * MOD * D      # rows of per-core x (3072)
OUT_ROWS = BP * N_NODES        # 16384
RREP = 16                      # rows per partition per init block (2MB blocks)
NBLK = OUT_ROWS // (P * RREP)  # 8 init blocks

# misc128 column layout
MC_TRI = 0          # [128,128] strict lower triangle (j>e mask)
MC_COEF = 128       # 12 coefficient vectors x NCH columns
MC_B1 = 128 + 12 * NCH          # [128,4] b1
MC_IDSF = MC_B1 + 4             # [128,NCH] float ids (column c = chunk c)
M128 = MC_IDSF + NCH            # 223
WW_SHIFT = 1024     # WEWA columns 1024:1408 = 3 shift matrices S_k = eye(k=-k)
WW = WW_SHIFT + 3 * 128
# misc1 column layout
M1_BEBA = 0         # be||ba [512]
M1_IDS = 512        # float ids row [S_C]
M1_INIT = 512 + S_C  # init_mem [256]
M1 = M1_INIT + MEM   # 1664

_NC_CACHE = {}


def _build_nc():
    """Build the single-core Bass/Tile program (SPMD across the 8 cores)."""
    nc = bacc.Bacc("TRN2", target_bir_lowering=False, debug=False,
                   enable_asserts=False, num_devices=N_CORES)
    t = {}
    t["xT0"] = nc.dram_tensor("xT0", [P, S_C], F32, kind="ExternalInput").ap()
    t["xT1"] = nc.dram_tensor("xT1", [P, S_C], F32, kind="ExternalInput").ap()
    t["misc128"] = nc.dram_tensor("misc128", [P, M128], F32, kind="ExternalInput").ap()
    t["misc1"] = nc.dram_tensor("misc1", [1, M1], F32, kind="ExternalInput").ap()
    t["misc64"] = nc.dram_tensor("misc64", [DEMO, 517], F32, kind="ExternalInput").ap()
    t["W2P"] = nc.dram_tensor("W2P", [P, 256], F32, kind="ExternalInput").ap()
    t["W3B"] = nc.dram_tensor("W3B", [DEMO + 1, MEM], F32, kind="ExternalInput").ap()
    t["WEWA"] = nc.dram_tensor("WEWA", [P, WW], F32, kind="ExternalInput").ap()
    t["out"] = nc.dram_tensor("out", [OUT_ROWS, MEM], F32, kind="ExternalOutput").ap()
    t["rows"] = nc.dram_tensor("rows", [S_C + BP, MEM], F32, kind="ExternalOutput").ap()

    with tile.TileContext(nc) as tc:
        with ExitStack() as ctx:
            _emit(ctx, tc, **t)
    nc.compile()
    return nc


def _emit(ctx, tc, *, xT0, xT1, misc128, misc1, misc64, W2P, W3B, WEWA, out, rows):
    nc = tc.nc

    const = ctx.enter_context(tc.tile_pool(name="const", bufs=1))
    big = ctx.enter_context(tc.tile_pool(name="big", bufs=1))
    work = ctx.enter_context(tc.tile_pool(name="work", bufs=4))
    psum = ctx.enter_context(tc.tile_pool(name="psum", bufs=1, space="PSUM"))

    # ---------- loads (SP carries the later init-block writes; reads that
    # gate compute split across SP/Act so neither queue stalls) ----------
    xT = [big.tile([P, S_C], R32, tag=f"xT{i}", name=f"xT{i}") for i in range(2)]
    nc.sync.dma_start(xT[0][:], xT0.bitcast(R32))
    nc.scalar.dma_start(xT[1][:], xT1.bitcast(R32))
    m128 = const.tile([P, M128], F32, tag="m128", name="m128")
    nc.sync.dma_start(m128[:], misc128[:])
    m1 = const.tile([1, M1], F32, tag="m1", name="m1")
    nc.scalar.dma_start(m1[:], misc1[:])
    beba = const.tile([1, 512], R32, tag="beba", name="beba")
    nc.scalar.dma_start(beba[:], misc1[:, M1_BEBA:M1_BEBA + 512].bitcast(R32))
    m64 = const.tile([DEMO, 517], R32, tag="m64", name="m64")
    nc.sync.dma_start(m64[:], misc64.bitcast(R32))
    w2 = const.tile([P, 256], R32, tag="w2", name="w2")
    nc.scalar.dma_start(w2[:], W2P.bitcast(R32))
    w3b = const.tile([DEMO + 1, MEM], R32, tag="w3b", name="w3b")
    nc.sync.dma_start(w3b[:], W3B.bitcast(R32))
    wewa = const.tile([P, WW], R32, tag="wewa", name="wewa")
    nc.scalar.dma_start(wewa[:], WEWA.bitcast(R32))

    # ---------- derived constants ----------
    ones_f = const.tile([1, P], F32, tag="ones_f", name="ones_f")
    nc.vector.memset(ones_f[:], 1.0)
    ones_row = const.tile([1, P], R32, tag="ones_row", name="ones_row")
    nc.vector.tensor_copy(ones_row[:], ones_f[:])
    init128 = const.tile([P, MEM], F32, tag="init128", name="init128")
    nc.gpsimd.partition_broadcast(init128[:], m1[:, M1_INIT:M1_INIT + MEM])
    ids_row128 = const.tile([P, S_C], F32, tag="ids_row128", name="ids_row128")
    nc.gpsimd.partition_broadcast(ids_row128[:], m1[:, M1_IDS:M1_IDS + S_C])

    def coef(v, c):
        i = MC_COEF + v * NCH + c
        return m128[:, i:i + 1]

    tri_f = m128[:, MC_TRI:MC_TRI + P]

    # initrep[p, r*MEM+m] = init_mem[m]: doubling copies on compute engines
    # (not DMA - the cost of SBUF->SBUF DMA competes with the HBM writes).
    initrep = big.tile([P, RREP * MEM], F32, tag="initrep", name="initrep")
    nc.vector.tensor_copy(initrep[:, 0:MEM], init128[:])
    copy_engs = (nc.vector, nc.gpsimd, nc.vector, nc.gpsimd)
    w = MEM
    i = 0
    while w < RREP * MEM:
        copy_engs[i].tensor_copy(initrep[:, w:2 * w], initrep[:, 0:w])
        w *= 2
        i += 1

    # the full init table: 8 x 2MB writes on the SP queue
    for blk in range(NBLK):
        dst = out[blk * P * RREP:(blk + 1) * P * RREP, :].rearrange(
            "(p r) m -> p (r m)", r=RREP)
        nc.sync.dma_start(dst, initrep[:])

    # ---------- demographics residual block (tiny, feeds rows[S_C:]) ----------
    hT = [const.tile([P, BP], R32, tag=f"hT{i}", name=f"hT{i}") for i in range(4)]
    demoT = m64[:, 512:516]
    for i in range(4):
        ps = psum.tile([P, BP], F32, tag="pA", bufs=2, name="demo_ps")
        nc.tensor.matmul(ps[:], lhsT=m64[:, i * P:(i + 1) * P],
                         rhs=demoT, start=True, stop=True)
        nc.scalar.activation(hT[i][:], ps[:], AF.Relu,
                             bias=m128[:, MC_B1 + i:MC_B1 + i + 1], scale=1.0)
    ps_y = psum.tile([DEMO, BP], F32, tag="pB", bufs=2, name="demo_y")
    for i in range(4):
        nc.tensor.matmul(ps_y[:], lhsT=w2[:, i * DEMO:(i + 1) * DEMO],
                         rhs=hT[i][:], start=(i == 0), stop=(i == 3))
    yTe = const.tile([DEMO + 1, BP], R32, tag="yTe", name="yTe")
    nc.vector.tensor_copy(yTe[DEMO:DEMO + 1, :], ones_f[:, 0:BP])
    nc.scalar.activation(yTe[0:DEMO, :], ps_y[:], AF.Identity,
                         bias=m64[:, 516:517].bitcast(F32), scale=1.0)
    nc.vector.tensor_add(yTe[0:DEMO, :], yTe[0:DEMO, :].bitcast(F32),
                         demoT.bitcast(F32))
    psde = psum.tile([BP, MEM], F32, tag="pC", bufs=2, name="demo_de")
    nc.tensor.matmul(psde[:], lhsT=yTe[:], rhs=w3b[:],
                     start=True, stop=True)
    de_s = work.tile([BP, MEM], F32, tag="de", name="de")
    nc.vector.tensor_copy(de_s[:], psde[:])

    # ---------- main pipeline over the 7 event chunks ----------
    AlT = big.tile([P, NCH * MEM], F32, tag="AlT", name="AlT")
    Bf = big.tile([P, NCH * MEM], F32, tag="Bf", name="Bf")
    G = big.tile([P, NCH * S_C], R32, tag="G", name="G")
    Gd = big.tile([P, NCH * P], R32, tag="Gd", name="Gd")
    Gd2 = big.tile([P, NCH * P], R32, tag="Gd2", name="Gd2")
    lnAf = big.tile([P, NCH * MEM], R32, tag="lnAf", name="lnAf")
    contrib = big.tile([P, NCH * MEM], R32, tag="contrib", name="contrib")
    eAll_t = big.tile([P, NCH * MEM], F32, tag="eAll_t", name="eAll_t")
    rowsAll = big.tile([P, NCH * MEM], F32, tag="rowsAll", name="rowsAll")

    def cc(c, w):
        return slice(c * w, (c + 1) * w)

    for c in range(NCH):
        # E/A gate matmuls (event-major out), bias via rank-1 matmul
        psEA = psum.tile([P, 2 * MEM], F32, tag="pA", bufs=2, name="psEA")
        nc.tensor.matmul(psEA[:], lhsT=ones_row[:],
                         rhs=beba[:],
                         start=True, stop=False)
        for i in range(2):
            nc.tensor.matmul(psEA[:], lhsT=xT[i][:, cc(c, P)],
                             rhs=wewa[:, i * 512:(i + 1) * 512],
                             start=False, stop=(i == 1))
        thA = work.tile([P, 2 * MEM], R32, tag="thA", name="thA")
        nc.scalar.activation(thA[:, 0:MEM], psEA[:, 0:MEM], AF.Tanh, scale=0.5)
        nc.scalar.activation(thA[:, MEM:2 * MEM], psEA[:, MEM:2 * MEM], AF.Tanh)
        th_f = thA[:, 0:MEM].bitcast(F32)
        A_f = thA[:, MEM:2 * MEM].bitcast(F32)

        # D-level chain: partition shifts via PE shift-identity matmuls,
        # coefficients pre-shifted on host so everything stays row-local.
        psh = []
        for k in (1, 2, 3):
            ps = psum.tile([P, 2 * MEM], F32, tag="pBCD"[0] + "BCD"[k - 1],
                           bufs=2, name=f"sh{k}")
            nc.tensor.matmul(
                ps[:], lhsT=wewa[:, WW_SHIFT + (k - 1) * P:WW_SHIFT + k * P],
                rhs=thA[:], start=True, stop=True)
            psh.append(ps)
        Al = AlT[:, cc(c, MEM)]
        Bc = Bf[:, cc(c, MEM)]
        nc.vector.tensor_scalar(Al, th_f, coef(1, c), coef(2, c),
                                op0=OP.mult, op1=OP.add)
        nc.vector.tensor_scalar_mul(Bc, A_f, coef(0, c))
        for k in (1, 2, 3):
            ps = psh[k - 1]
            Mk = work.tile([P, MEM], F32, tag=f"Mk{k}", name=f"Mk{k}")
            nc.vector.tensor_scalar(Mk[:], ps[:, 0:MEM], coef(3 * k, c),
                                    coef(3 * k + 1, c), op0=OP.mult, op1=OP.add)
            Ms = work.tile([P, MEM], F32, tag=f"Ms{k}", name=f"Ms{k}")
            nc.vector.tensor_scalar_mul(Ms[:], ps[:, MEM:2 * MEM], coef(3 * k + 2, c))
            nc.vector.tensor_mul(Al, Al, Mk[:])
            nc.gpsimd.tensor_tensor(Bc, Bc, Mk[:], op=OP.mult)
            nc.gpsimd.tensor_tensor(Bc, Bc, Ms[:], op=OP.add)
        nc.vector.tensor_scalar_max(Al, Al, 1e-30)

        # compare-matrix rows for this j-chunk
        nc.vector.tensor_tensor(G[:, cc(c, S_C)],
                                m128[:, MC_IDSF + c:MC_IDSF + c + 1].to_broadcast([P, S_C]),
                                ids_row128[:], op=OP.is_equal)
        diag = G[:, c * S_C + c * P: c * S_C + (c + 1) * P]
        nc.vector.tensor_mul(Gd[:, cc(c, P)], diag, tri_f)
        nc.vector.tensor_tensor(Gd2[:, cc(c, P)], diag, Gd[:, cc(c, P)],
                                op=OP.subtract)

    # ln in two batches: early chunks' ln unblocks PE sooner
    nc.scalar.activation(lnAf[:, 0:4 * MEM], AlT[:, 0:4 * MEM], AF.Ln)
    nc.scalar.activation(lnAf[:, 4 * MEM:], AlT[:, 4 * MEM:], AF.Ln)

    # ---------- suffix/total G matmuls, contrib, rows ----------
    def gblk(J, E_):
        return G[:, J * S_C + E_ * P: J * S_C + (E_ + 1) * P]

    for E_ in range(NCH):
        ps = psum.tile([P, MEM], F32, tag=("pB" if E_ % 2 else "pA"),
                       bufs=2, name="psSuf")
        js = sorted(range(E_, NCH), key=lambda j: (j >= 4, j))
        for n_, J in enumerate(js):
            lhsT = Gd[:, cc(E_, P)] if J == E_ else gblk(J, E_)
            nc.tensor.matmul(ps[:], lhsT=lhsT, rhs=lnAf[:, cc(J, MEM)],
                             start=(n_ == 0), stop=(n_ == len(js) - 1))
        eSuf = work.tile([P, MEM], F32, tag="eSuf", name="eSuf")
        nc.scalar.activation(eSuf[:], ps[:], AF.Exp)
        nc.gpsimd.tensor_tensor(contrib[:, cc(E_, MEM)], Bf[:, cc(E_, MEM)],
                                eSuf[:], op=OP.mult)
        # prefix (j<=e) continues into the same bank -> AllLog for free
        for J in range(0, E_ + 1):
            lhsT = Gd2[:, cc(E_, P)] if J == E_ else gblk(J, E_)
            nc.tensor.matmul(ps[:], lhsT=lhsT, rhs=lnAf[:, cc(J, MEM)],
                             start=False, stop=(J == E_), skip_group_check=True)
        nc.scalar.activation(eAll_t[:, cc(E_, MEM)], ps[:], AF.Exp)
    for E_ in range(NCH):
        psB = psum.tile([P, MEM], F32, tag=("pD" if E_ % 2 else "pC"),
                        bufs=2, name="psB")
        for J in range(NCH):
            nc.tensor.matmul(psB[:], lhsT=gblk(J, E_),
                             rhs=contrib[:, cc(J, MEM)],
                             start=(J == 0), stop=(J == NCH - 1))
        r = rowsAll[:, cc(E_, MEM)]
        nc.gpsimd.tensor_tensor(r, eAll_t[:, cc(E_, MEM)], init128[:], op=OP.mult)
        nc.vector.tensor_add(r, r, psB[:])

    # dense row buffer: events then demo rows; host does final placement
    nc.sync.dma_start(rows[S_C:S_C + BP, :], de_s[:])
    nc.sync.dma_start(rows[0:S_C, :].rearrange("(c p) m -> p c m", p=P),
                      rowsAll[:].rearrange("p (c m) -> p c m", c=NCH))


def _assign_patients(gvalid):
    """Balanced 4-patients-per-core assignment by valid-group count (LPT)."""
    counts = gvalid.reshape(B, -1).sum(1)
    order = np.argsort(-counts, kind="stable")
    loads = [0] * N_CORES
    members = [[] for _ in range(N_CORES)]
    for p in order:
        c = min((c for c in range(N_CORES) if len(members[c]) < BP),
                key=lambda c: loads[c])
        members[c].append(int(p))
        loads[c] += int(counts[p])
    assert max(loads) * D <= S_C, f"core load {max(loads)} groups > {S_C // D}"
    return members


def _host_prep(inputs):
    """Index-only host prep: compaction, balancing, index/coefficient tensors."""
    x = np.ascontiguousarray(np.asarray(inputs["input"], np.float32)).reshape(B, T * MOD * D, WD)
    mask = np.asarray(inputs["mask"])
    valid_mod = np.asarray(inputs["valid_mod"])
    node_ids = np.asarray(inputs["node_ids"])
    demo = np.ascontiguousarray(np.asarray(inputs["demo"], np.float32))

    W1 = np.asarray(inputs["W1"], np.float32)
    b1 = np.asarray(inputs["b1"], np.float32)
    W2 = np.asarray(inputs["W2"], np.float32)
    b2 = np.asarray(inputs["b2"], np.float32)
    W3 = np.asarray(inputs["W3"], np.float32)
    b3 = np.asarray(inputs["b3"], np.float32)
    We = np.asarray(inputs["We"], np.float32)
    be = np.asarray(inputs["be"], np.float32)
    Wa = np.asarray(inputs["Wa"], np.float32)
    ba = np.asarray(inputs["ba"], np.float32)
    init_mem = np.asarray(inputs["init_mem"], np.float32)

    m128_base = np.zeros((P, M128), np.float32)
    m128_base[:, MC_TRI:MC_TRI + P] = np.tril(np.ones((P, P), np.float32), -1)
    m128_base[:, MC_B1:MC_B1 + 4] = b1.reshape(4, P).T

    m1_base = np.zeros((1, M1), np.float32)
    m1_base[0, M1_BEBA:M1_BEBA + MEM] = be
    m1_base[0, M1_BEBA + MEM:M1_BEBA + 2 * MEM] = ba
    m1_base[0, M1_INIT:M1_INIT + MEM] = init_mem

    W2P = np.ascontiguousarray(
        W2.reshape(4, P, DEMO).transpose(1, 0, 2).reshape(P, 4 * DEMO))
    W3B = np.ascontiguousarray(np.concatenate([W3, b3[None, :]], axis=0))
    WEWA = np.zeros((P, WW), np.float32)
    WEWA[:, 0:1024] = np.concatenate(
        [We.reshape(2, P, MEM), Wa.reshape(2, P, MEM)],
        axis=2).transpose(1, 0, 2).reshape(P, 1024)
    for k in (1, 2, 3):
        WEWA[:, WW_SHIFT + (k - 1) * P:WW_SHIFT + k * P] = np.eye(
            P, k=-k, dtype=np.float32)

    gvalid = (mask[:, :, None] > 0) & (valid_mod > 0)   # [B, T, MOD]
    members = _assign_patients(gvalid)

    # shifted-coefficient masks (constant across cores except for vf)
    dpat = np.arange(P) % 4

    in_maps = []
    scat = []
    for core in range(N_CORES):
        pats = members[core]
        xg = np.zeros((S_C,), np.int32)
        idsv = np.full((S_C,), 1, np.int32)     # pads -> slot0 node 1 (benign)
        vf = np.zeros((S_C,), np.float32)
        e = 0
        for slot, b in enumerate(pats):
            tms = np.nonzero(gvalid[b].reshape(T * MOD))[0]
            for tm in tms:
                for d in range(D):
                    xg[e] = slot * (T * MOD * D) + tm * D + d
                    idsv[e] = slot * N_NODES + int(
                        node_ids[b, tm // MOD, tm % MOD, d])
                    vf[e] = 1.0
                    e += 1
        xe = x[pats].reshape(X_ROWS, WD)[xg].T     # [WD, S_C]

        vf2 = np.ascontiguousarray(vf.reshape(NCH, P).T)   # [128, NCH]
        co = np.zeros((P, 12 * NCH), np.float32)
        co[:, 0 * NCH:1 * NCH] = vf2                       # val
        co[:, 1 * NCH:2 * NCH] = -vf2 / 2                  # nvA
        co[:, 2 * NCH:3 * NCH] = 1 - vf2 / 2               # nvB
        for k in (1, 2, 3):
            vsh = np.zeros((P, NCH), np.float32)
            vsh[0:P - k, :] = vf2[k:P, :]
            msk = np.zeros((P, 1), np.float32)
            msk[0:P - k, 0] = (dpat[k:P] >= k).astype(np.float32)
            ck = 2.0 ** -k
            ca = -(ck / 2) * msk * vsh
            co[:, (3 * k) * NCH:(3 * k + 1) * NCH] = ca
            co[:, (3 * k + 1) * NCH:(3 * k + 2) * NCH] = 1.0 + ca
            co[:, (3 * k + 2) * NCH:(3 * k + 3) * NCH] = ck * msk * vsh

        m128c = m128_base.copy()
        m128c[:, MC_COEF:MC_COEF + 12 * NCH] = co
        m128c[:, MC_IDSF:MC_IDSF + NCH] = idsv.astype(np.float32).reshape(NCH, P).T
        m1c = m1_base.copy()
        m1c[0, M1_IDS:M1_IDS + S_C] = idsv.astype(np.float32)
        m64c = np.zeros((DEMO, 517), np.float32)
        m64c[:, 0:512] = W1
        m64c[:, 512:516] = demo[pats].T
        m64c[:, 516] = b2

        in_maps.append({
            "xT0": np.ascontiguousarray(xe[0:P]),
            "xT1": np.ascontiguousarray(xe[P:2 * P]),
            "misc128": m128c, "misc1": m1c, "misc64": m64c,
            "W2P": W2P, "W3B": W3B, "WEWA": WEWA,
        })
        scat.append((idsv, vf))
    return in_maps, members, scat


def _assemble(res, members, scat):
    out = np.empty((B, N_NODES, MEM), np.float32)
    for core in range(N_CORES):
        r = res.results[core]
        idsv, vf = scat[core]
        block = np.array(r["out"]).reshape(BP * N_NODES, MEM)
        rows = np.asarray(r["rows"])
        ev = vf > 0
        block[idsv[ev]] = rows[:S_C][ev]
        blk4 = block.reshape(BP, N_NODES, MEM)
        for slot, b in enumerate(members[core]):
            out[b] = blk4[slot]
            out[b, 0] = rows[S_C + slot]
    return out


def get_nc():
    if "nc" not in _NC_CACHE:
        _NC_CACHE["nc"] = _build_nc()
    return _NC_CACHE["nc"]


def run_cores(inputs, trace=False):
    nc = get_nc()
    in_maps, members, scat = _host_prep(inputs)
    res = bass_utils.run_bass_kernel_spmd(
        nc, in_maps, core_ids=list(range(N_CORES)), trace=trace)
    return _assemble(res, members, scat), res


def kernel(**inputs) -> np.ndarray:
    return run_cores(inputs)[0]


if __name__ == "__main__":
    ref = {}
    exec(open("/root/problem/reference.py").read(), ref)
    inputs = {k: np.asarray(v) for k, v in ref["setup_inputs"]().items()}
    got = kernel(**inputs)
    want = np.asarray(ref["reference"](**inputs))
    err = np.abs(got - want).max() / np.abs(want).max()
    print("rel err:", err)
